# revision 24
# baseline (speedup 1.0000x reference)
"""Distributed causal MHA for TRN2 (8 NeuronCores), v6.

Core c: batch c//2, parity par=c%2. Queries split into 16 slots of 64
(slot k = 64-token block 2k+par), so slot k needs key tiles 0..k exactly
on BOTH parities -> zero causal padding and an identical shared graph.

All per-dim-slice inputs (xTq | wqk | xT | wv | wo) are host-packed into
one [1024, 7168] bf16 tensor -> 8 big DMAs load everything.

Attention per head-pair hp runs in two 512-query passes (pass 0: slots
0-7 / key tiles 0-7; pass 1: slots 8-15 / key tiles 0-15). Per key tile
one wide score matmul per head into a double-buffered [128,1024] PSUM
tile (h0 cols 0-511, h1 cols 512-1023), ONE exp on ACT spanning both
heads via a 2-segment AP, a [128,64] DVE mask on the diagonal block,
and AV accumulation into per-head [65,512] PSUM accumulators with a
ones-column in V producing denominators. Normalization on-chip:
fast-approx reciprocal, PE broadcast, DVE multiply. Q/K/V projections
for pair hp+1 are interleaved into pair hp's attention stream as PE
filler to keep HAM warm.
"""

import sys
from collections import deque

sys.path.insert(0, "/opt/trn_rl_repo")
import numpy as np
import ml_dtypes
import concourse.bass as bass
import concourse.mybir as mybir
import concourse.tile as tile
from concourse.vector_clock import ScopedClock
from concourse.bass_utils import run_bass_kernel_spmd

B, N, DIM = 4, 2048, 1024
HEADS, DH = 16, 64
INNER = HEADS * DH
SCALE = DH ** -0.5
NQ = 1024            # queries per core
NSLOT = 16           # 64-query slots per core
NKT = 16             # 128-key tiles
PCOLS = 4096         # packed input cols: xTq 1024 | xT 2048 | wo 1024
F32 = mybir.dt.float32
BF16 = mybir.dt.bfloat16
AF = mybir.ActivationFunctionType
ALU = mybir.AluOpType

LAST_RESULT = None


def _drain_and_barrier_patched(self, tick_clock, wait_clock):
    nop_inst = self.nc.sync.nop(nofuse=True)
    wait_clock.add_sem_waits(nop_inst.ins, ScopedClock({None: tick_clock.global_clock}))
    si = nop_inst.ins.sync_info
    waits = list(si.on_wait or []) if si else []
    if len(waits) > 1:
        nop_inst.ins.sync_info = mybir.SyncInfo(
            on_wait=waits[:1], on_update=list(si.on_update or [])
        )
        for i in range(1, len(waits)):
            extra = self.nc.sync.nop(nofuse=True)
            extra.ins.sync_info = mybir.SyncInfo(on_wait=[waits[i]], on_update=[])
    self.nc.sync.drain()
    self.nc.all_engine_barrier()
    popped = self.nc._tile_sem_poison_stack.pop()
    assert popped is self._sem_poison
    self.nc.clear_and_free_semaphores(list(self.sems.allocated().values()))
    self.nc.all_engine_barrier()


tile.TileContext._drain_and_barrier = _drain_and_barrier_patched


def _split_multi_waits(nc):
    for f in nc.m.functions:
        for bb in f.blocks:
            insts = bb.instructions
            if not any(
                i.sync_info and i.sync_info.on_wait and len(i.sync_info.on_wait) > 1
                for i in insts
            ):
                continue
            new = []
            for inst in insts:
                si = inst.sync_info
                waits = list(si.on_wait) if si and si.on_wait else []
                if len(waits) > 1:
                    for w in waits[:-1]:
                        nop = mybir.InstNoOp(
                            name=nc.get_next_instruction_name(), ins=[], outs=[]
                        )
                        nop.engine = inst.engine
                        nop.sync_info = mybir.SyncInfo(on_wait=[w], on_update=[])
                        new.append(nop)
                    inst.sync_info = mybir.SyncInfo(
                        on_wait=[waits[-1]], on_update=list(si.on_update or [])
                    )
                new.append(inst)
            bb.instructions = new


def build_graph():
    nc = bass.Bass("TRN2", target_bir_lowering=False)

    p_inp = nc.declare_dram_parameter("inp", [DIM, PCOLS], BF16, isOutput=False)
    p_wqk = nc.declare_dram_parameter("wqk", [8 * DIM, 256], BF16, isOutput=False)
    p_wv = nc.declare_dram_parameter("wv", [4 * DIM, 256], BF16, isOutput=False)
    p_wbias = nc.declare_dram_parameter("wbias", [1, DIM], BF16, isOutput=False)
    p_mask = nc.declare_dram_parameter("maskt", [128, 128], BF16, isOutput=False)
    p_out = nc.declare_dram_parameter("out", [NQ, DIM], F32, isOutput=True)

    with tile.TileContext(nc) as tc:
        cst = tc.alloc_tile_pool(name="const", bufs=1)
        inpp = tc.alloc_tile_pool(name="inp", bufs=1)
        qtp = tc.alloc_tile_pool(name="qt", bufs=1)
        ktrp = tc.alloc_tile_pool(name="ktr", bufs=1)
        vsp = tc.alloc_tile_pool(name="vsb", bufs=1)
        afp = tc.alloc_tile_pool(name="af", bufs=1)
        wqkp = tc.alloc_tile_pool(name="wqk", bufs=2)
        wvp = tc.alloc_tile_pool(name="wv2", bufs=2)
        ewp = tc.alloc_tile_pool(name="ew", bufs=5)
        owp = tc.alloc_tile_pool(name="ow", bufs=3)
        rcpp = tc.alloc_tile_pool(name="rcp", bufs=1)

        # ---------- packed inputs: 8 big DMAs ----------
        big = [inpp.tile([128, PCOLS], BF16, tag=f"big{i}", name=f"big{i}")
               for i in range(8)]
        dma_engs = [nc.sync, nc.scalar, nc.gpsimd]
        xtq = [big[i][:, 0:1024] for i in range(8)]
        xt = [big[i][:, 1024:3072] for i in range(8)]
        wo = [big[i][:, 3072:4096] for i in range(8)]

        def load_wqk(hp, eng=None):
            eng = eng or nc.sync
            ts = [wqkp.tile([128, 256], BF16, tag=f"wqk{kt}", name=f"wqk{kt}_{hp}")
                  for kt in range(8)]
            for kt in range(8):
                eng.dma_start(
                    ts[kt][:, :], p_wqk[hp * DIM + kt * 128: hp * DIM + (kt + 1) * 128, :]
                )
            return ts

        def load_wv2(g, eng=None):
            eng = eng or nc.sync
            ts = [wvp.tile([128, 256], BF16, tag=f"wv2{kt}", name=f"wv2{kt}_{g}")
                  for kt in range(8)]
            for kt in range(8):
                eng.dma_start(
                    ts[kt][:, :], p_wv[g * DIM + kt * 128: g * DIM + (kt + 1) * 128, :]
                )
            return ts

        wqk_t = {0: load_wqk(0, nc.sync)}
        wv2_t = {0: load_wv2(0, nc.gpsimd)}
        maskt = cst.tile([128, 128], BF16, tag="maskt", name="maskt")
        nc.scalar.dma_start(maskt[:, :], p_mask[:, :])
        # bulk inputs: query columns first (QT runs first), then xT, wo last
        for i in range(8):
            dma_engs[i % 3].dma_start(
                big[i][:, 0:512], p_inp[i * 128:(i + 1) * 128, 0:512]
            )
        for i in range(8):
            dma_engs[i % 3].dma_start(
                big[i][:, 512:1024], p_inp[i * 128:(i + 1) * 128, 512:1024]
            )
        for i in range(8):
            dma_engs[i % 3].dma_start(
                big[i][:, 1024:2048], p_inp[i * 128:(i + 1) * 128, 1024:2048]
            )
        for i in range(8):
            dma_engs[i % 3].dma_start(
                big[i][:, 2048:3072], p_inp[i * 128:(i + 1) * 128, 2048:3072]
            )
        wqk_t[1] = load_wqk(1, nc.scalar)
        for i in range(8):
            dma_engs[i % 3].dma_start(
                big[i][:, 3072:4096], p_inp[i * 128:(i + 1) * 128, 3072:4096]
            )

        def wq(hp, kt):
            return wqk_t[hp][kt][:, 0:128]

        def wk(hp, kt):
            return wqk_t[hp][kt][:, 128:256]

        def wv2(g, kt):
            return wv2_t[g][kt][:, :]

        wbias = cst.tile([1, DIM], BF16, tag="wbias", name="wbias")
        nc.sync.dma_start(wbias[:, :], p_wbias[:, :])
        ones64 = cst.tile([1, 64], F32, tag="ones64", name="ones64")
        nc.vector.memset(ones64[:, :], 1.0)
        onesb = cst.tile([1, 1024], BF16, tag="onesb", name="onesb")
        nc.vector.memset(onesb[:, :], 1.0)

        qt = [qtp.tile([128, NQ], BF16, tag=f"qt{i}", name=f"qt{i}") for i in range(8)]
        ktr = [ktrp.tile([128, N], BF16, tag=f"kt{i}", name=f"kt{i}") for i in range(8)]
        vsb = [vsp.tile([128, HEADS * (DH + 1)], BF16, tag=f"v{i}", name=f"v{i}") for i in range(16)]
        for tt in range(16):
            nc.vector.memset(
                vsb[tt][:, :].rearrange("p (g d) -> p g d", g=16)[:, :, 64:65], 1.0
            )
        af = [afp.tile([128, NQ], BF16, tag=f"af{i}", name=f"af{i}") for i in range(8)]

        with (
            tc.tile_pool(name="psS", bufs=2, space="PSUM") as psS,
            tc.tile_pool(name="psA", bufs=1, space="PSUM") as psA,
            tc.tile_pool(name="psP", bufs=2, space="PSUM") as psP,
        ):
            # ---------- projection emitters ----------
            def qt_chunk(hp, tc2):
                pq = psP.tile([128, 512], F32, tag="proj", name="pq")
                for kt in range(8):
                    nc.tensor.matmul(
                        pq[:, :],
                        wq(hp, kt),
                        xtq[kt][:, tc2 * 512:(tc2 + 1) * 512],
                        start=(kt == 0),
                        stop=(kt == 7),
                    )
                nc.vector.tensor_copy(qt[hp][:, tc2 * 512:(tc2 + 1) * 512], pq[:, :])

            def kt_chunk(hp, tc4):
                pk = psP.tile([128, 512], F32, tag="proj", name="pk")
                for kt in range(8):
                    nc.tensor.matmul(
                        pk[:, :],
                        wk(hp, kt),
                        xt[kt][:, tc4 * 512:(tc4 + 1) * 512],
                        start=(kt == 0),
                        stop=(kt == 7),
                    )
                nc.vector.tensor_copy(ktr[hp][:, tc4 * 512:(tc4 + 1) * 512], pk[:, :])

            def v_chunk(g, tt, half=None):
                # half=None: both pairs of group g; half=0/1: one pair only
                c0, cw = (0, 256) if half is None else (128 * half, 128)
                pv = psP.tile([128, 256], F32, tag="proj", name="pv")
                for kt in range(8):
                    nc.tensor.matmul(
                        pv[:, 0:cw],
                        xt[kt][:, tt * 128:(tt + 1) * 128],
                        wv2(g, kt)[:, c0:c0 + cw],
                        start=(kt == 0),
                        stop=(kt == 7),
                    )
                ne = 4 if half is None else 2
                dst = vsb[tt][:, g * 260 + 130 * (half or 0):
                              g * 260 + 130 * (half or 0) + 65 * ne].rearrange(
                    "p (e d) -> p e d", e=ne
                )[:, :, 0:64]
                nc.vector.tensor_copy(
                    dst, pv[:, 0:cw].rearrange("p (e d) -> p e d", e=ne)
                )

            def out_group(it, oc):
                po = psP.tile([128, 512], F32, tag="proj", name="po")
                for ft in range(8):
                    nc.tensor.matmul(
                        po[:, :],
                        af[ft][:, it * 128:(it + 1) * 128],
                        wo[ft][:, oc * 512:(oc + 1) * 512],
                        start=(ft == 0),
                        stop=False,
                    )
                nc.tensor.matmul(
                    po[:, :],
                    onesb[:, it * 128:(it + 1) * 128],
                    wbias[:, oc * 512:(oc + 1) * 512],
                    start=False,
                    stop=True,
                )
                ot = owp.tile([128, 512], F32, tag="ot", name="ot")
                nc.vector.tensor_copy(ot[:, :], po[:, :])
                nc.sync.dma_start(
                    p_out[it * 128:(it + 1) * 128, oc * 512:(oc + 1) * 512],
                    ot[:, :],
                )

            # ---------- prologue: minimum for attention(0) to start ----------
            for tc2 in range(2):
                qt_chunk(0, tc2)
            kt_chunk(0, 0)
            for tt in range(4):
                v_chunk(0, tt)

            # ---------- attention with interleaved projections ----------
            for hp in range(8):
                h0, h1 = 2 * hp, 2 * hp + 1
                if hp + 2 < 8:
                    wqk_t[hp + 2] = load_wqk(hp + 2)
                if hp % 2 == 0 and (hp + 2) // 2 < 4:
                    g2 = (hp + 2) // 2
                    wv2_t[g2] = load_wv2(g2)

                filler = deque()
                if hp == 0:
                    # leftovers of pair 0's own K/V, ordered by first use:
                    # scores jt needs KT chunk jt//4; AV jt needs vsb[jt]
                    filler.append((kt_chunk, (0, 1)))
                    for tt in range(4, 7):
                        filler.append((v_chunk, (0, tt)))
                    filler.append((kt_chunk, (0, 2)))
                    for tt in range(7, 10):
                        filler.append((v_chunk, (0, tt)))
                    filler.append((kt_chunk, (0, 3)))
                    for tt in range(10, 16):
                        filler.append((v_chunk, (0, tt)))
                if hp + 1 < 8:
                    for tc2 in range(2):
                        filler.append((qt_chunk, (hp + 1, tc2)))
                    for tc4 in range(4):
                        filler.append((kt_chunk, (hp + 1, tc4)))
                    if hp % 2 == 1 and hp < 5:
                        g = (hp + 1) // 2
                        for tt in range(16):
                            filler.append((v_chunk, (g, tt)))
                    if hp == 5:
                        for tt in range(16):
                            filler.append((v_chunk, (3, tt, 0)))
                    if hp == 6:
                        for tt in range(16):
                            filler.append((v_chunk, (3, tt, 1)))

                def pump(n=1):
                    for _ in range(n):
                        if filler:
                            fn, args = filler.popleft()
                            fn(*args)

                for pss in range(2):
                    if hp == 7 and pss == 1:
                        for it in range(4):
                            for oc in range(2):
                                filler.append((out_group, (it, oc)))
                    qbase = 512 * pss
                    acc = psA.tile([65, 1024], F32, tag="acc", name="acc")
                    jts = range(8) if pss == 0 else range(16)
                    njt = 8 if pss == 0 else 16

                    def emit_av(pjt, pa0, pW, peW):
                        for hi, h in ((0, h0), (1, h1)):
                            nc.tensor.matmul(
                                acc[:, 512 * hi + pa0:512 * hi + pa0 + pW],
                                vsb[pjt][:, h * 65:(h + 1) * 65],
                                peW[:, 512 * hi:512 * hi + pW],
                                start=(pjt == 0),
                                stop=(pjt == njt - 1),
                                skip_group_check=True,
                            )

                    pend = None
                    for jt in jts:
                        if pss == 0:
                            q0, W = 64 * jt, 512 - 64 * jt
                            diag = True
                        else:
                            q0 = max(512, 64 * jt)
                            W = 1024 - q0
                            diag = jt >= 8
                        S = psS.tile([128, 1024], F32, tag="S", name="S")
                        nc.tensor.matmul(
                            S[:, 0:W],
                            ktr[hp][0:64, jt * 128:(jt + 1) * 128],
                            qt[hp][0:64, q0:q0 + W],
                            start=True, stop=True,
                        )
                        nc.tensor.matmul(
                            S[:, 512:512 + W],
                            ktr[hp][64:128, jt * 128:(jt + 1) * 128],
                            qt[hp][64:128, q0:q0 + W],
                            start=True, stop=True,
                        )
                        eW = ewp.tile([128, 1024], BF16, tag="eW", name="eW")
                        s2 = S[:, :].rearrange("p (h w) -> p h w", h=2)[:, :, 0:W]
                        e2 = eW[:, :].rearrange("p (h w) -> p h w", h=2)[:, :, 0:W]
                        nc.scalar.activation(e2, s2, AF.Exp, scale=SCALE)
                        if diag:
                            ed = eW[:, :].rearrange("p (h w) -> p h w", h=2)[:, :, 0:64]
                            md = maskt[:, :].rearrange("p (g c) -> p g c", g=2)
                            nc.vector.tensor_mul(ed, ed, md)
                        if pend is not None:
                            emit_av(*pend)
                        pend = (jt, q0 - qbase, W, eW)
                        if not (hp == 7 and pss == 1 and jt < 4):
                            pump(1)
                    emit_av(*pend)
                    # ---- capture: ln(den) + unnormalized A -> af; frees acc fast
                    rcp = rcpp.tile([1, 1024], F32, tag="rcp", name="rcp")
                    nc.scalar.activation(rcp[:, :], acc[64:65, :], AF.Ln)
                    nc.vector.tensor_copy(
                        af[hp][0:64, qbase:qbase + 512], acc[0:64, 0:512]
                    )
                    nc.vector.tensor_copy(
                        af[hp][64:128, qbase:qbase + 512], acc[0:64, 512:1024]
                    )
                    # ---- normalize in place: af *= exp(-ln den) broadcast
                    nc.scalar.activation(rcp[:, :], rcp[:, :], AF.Exp, scale=-1.0)
                    rb = psS.tile([128, 1024], F32, tag="S", name="rb")
                    nc.tensor.matmul(
                        rb[0:64, 0:512], ones64[:, :], rcp[:, 0:512],
                        start=True, stop=True,
                    )
                    nc.tensor.matmul(
                        rb[64:128, 0:512], ones64[:, :], rcp[:, 512:1024],
                        start=True, stop=True,
                    )
                    rbs = rcpp.tile([128, 512], BF16, tag="rbs", name="rbs")
                    nc.vector.tensor_copy(rbs[:, :], rb[:, 0:512])
                    afh = af[hp]
                    nc.vector.tensor_mul(
                        afh[0:64, qbase:qbase + 512],
                        afh[0:64, qbase:qbase + 512],
                        rbs[0:64, :],
                    )
                    nc.vector.tensor_mul(
                        afh[64:128, qbase:qbase + 512],
                        afh[64:128, qbase:qbase + 512],
                        rbs[64:128, :],
                    )
                pump(len(filler))

            # ---------- remaining out-projection ----------
            for it in range(4, 8):
                for oc in range(2):
                    out_group(it, oc)

        for p in (rcpp, owp, ewp, wvp, wqkp, afp, vsp, ktrp, qtp, inpp, cst):
            p.release()

    _split_multi_waits(nc)
    return nc


_GRAPH = None


def _get_graph():
    global _GRAPH
    if _GRAPH is None:
        _GRAPH = build_graph()
    return _GRAPH


def kernel(x, mask, w_qkv, w_out, b_out):
    global LAST_RESULT
    x = np.asarray(x, dtype=np.float32)
    w_qkv = np.asarray(w_qkv, dtype=np.float32)
    w_out = np.asarray(w_out, dtype=np.float32)
    b_out = np.asarray(b_out, dtype=np.float32)

    nc = _get_graph()
    BF = ml_dtypes.bfloat16

    # streamed weights: wqk [8*1024, 256], wv [4*1024, 256]
    wqk = np.empty((8 * DIM, 256), np.float32)
    for hp in range(8):
        wqk[hp * DIM:(hp + 1) * DIM, 0:128] = w_qkv[:, 128 * hp:128 * (hp + 1)]
        wqk[hp * DIM:(hp + 1) * DIM, 128:256] = w_qkv[:, INNER + 128 * hp:INNER + 128 * (hp + 1)]
    wqk = np.ascontiguousarray(wqk.astype(BF))
    wv = np.ascontiguousarray(
        w_qkv[:, 2 * INNER:3 * INNER].T.reshape(4, 256, DIM).transpose(0, 2, 1).reshape(4 * DIM, 256).astype(BF)
    )
    wo_bf = w_out.astype(BF)
    wbias = np.ascontiguousarray(b_out[None, :].astype(BF))

    xT_bf = [x[b].T.astype(BF) for b in range(B)]

    p = np.arange(128)[:, None]
    r = np.arange(64)[None, :]
    in_maps = []
    for c in range(8):
        b, par = c // 2, c % 2
        qrows = np.concatenate(
            [np.arange(128 * k + 64 * par, 128 * k + 64 * par + 64) for k in range(NSLOT)]
        )
        inp = np.empty((DIM, PCOLS), BF)
        inp[:, 0:1024] = x[b][qrows].T.astype(BF)
        inp[:, 1024:3072] = xT_bf[b]
        inp[:, 3072:4096] = wo_bf
        m = (p <= (64 * par + r)).astype(np.float32)  # [128, 64]
        maskt = np.ascontiguousarray(np.concatenate([m, m], axis=1).astype(BF))
        in_maps.append(
            {
                "inp": inp,
                "wqk": wqk,
                "wv": wv,
                "wbias": wbias,
                "maskt": maskt,
            }
        )

    res = run_bass_kernel_spmd(nc, in_maps, list(range(8)))
    LAST_RESULT = res

    out = np.empty((B, N, DIM), dtype=np.float32)
    for c in range(8):
        b, par = c // 2, c % 2
        rr = res.results[c]["out"]
        for k in range(NSLOT):
            out[b, 128 * k + 64 * par:128 * k + 64 * par + 64] = rr[64 * k:64 * (k + 1)]
    return out
